# revision 19
# baseline (speedup 1.0000x reference)
"""Causal self-attention (B=4, T=2048, D=1024, H=16) on 8 trn2 NeuronCores.

Sharding: tensor-parallel over heads — 2 heads per core. Each core computes
qkv projections for its 2 heads (from replicated x), causal attention, and a
partial output projection (its 128 rows of w_proj). Host sums the 8 partial
[S, D] outputs.

v2: fine-grained software pipeline. The attention inner loop is ScalarE
latency-bound (exp of a [128,1024] score tile ~1.15us vs ~0.85us of PE work
per j-tile), so QKV-projection matmuls of batch b+1 and output-projection
matmuls of chunk (b,c-1) are interleaved between attention steps of batch b
at ~1-matmul-group granularity. This keeps TensorE busy through the
scalar-bound stretches, avoids >3.4us PE idle windows (which drop the HAM
clock gate from 2.4GHz to 1.2GHz), and removes the serial per-batch
normalize+project tail.

Other changes vs v1:
  - diagonal score matmuls N-sliced to the causal region (rhs[off:])
  - per-chunk softmax denominators: per-head DVE reciprocal_approx_fast on
    the [1,512] staging rows (exp/recip can't share the ScalarE table set,
    and InstReciprocal is 5x slower), then two K=1 broadcast matmuls into
    disjoint 64-col PE groups (they pair concurrently, like the K=64
    score-matmul pairs in PE row groups 0/64)
  - causal masks as one strided 2-head gpsimd multiply per diagonal tile
    (gpsimd's dedicated queue keeps them off the DVE/ScalarE critical paths)
  - x chunks prefetched one ahead; per-chunk coalesced x loads; per-tile
    output stores; PE warm-up matmuls cover the initial DMA latency
"""

import math
from collections import deque

import numpy as np
import ml_dtypes

B, T, D, H = 4, 2048, 1024, 16
HD = D // H           # 64
S = B * T             # 8192
P = 128
KT = D // P           # 8 k-tiles over D
NT = S // P           # 64 m-tiles of 128
JT = T // P           # 16 j-tiles per batch
NCH = T // 512        # 4 i-chunks per batch
N_CORES = 8

BFNP = ml_dtypes.bfloat16

_CACHE = {}


def _build_nc():
    import concourse.tile as tile
    import concourse.mybir as mybir
    from concourse import bacc

    BF = mybir.dt.bfloat16
    F32 = mybir.dt.float32
    F32R = mybir.dt.float32r
    Exp = mybir.ActivationFunctionType.Exp
    SCALE = 1.0 / math.sqrt(HD)

    nc = bacc.Bacc("TRN2", num_devices=N_CORES)

    xT = nc.dram_tensor("xT", [D, S], BF, kind="ExternalInput").ap()
    wq = nc.dram_tensor("wq", [D, P], BF, kind="ExternalInput").ap()
    wk = nc.dram_tensor("wk", [D, P], BF, kind="ExternalInput").ap()
    wv = nc.dram_tensor("wv", [D, P], BF, kind="ExternalInput").ap()
    wp = nc.dram_tensor("wp", [P, D], BF, kind="ExternalInput").ap()
    maskt = nc.dram_tensor("maskt", [P, P], BF, kind="ExternalInput").ap()
    e01 = nc.dram_tensor("e01", [2, P], BF, kind="ExternalInput").ap()
    ident = nc.dram_tensor("ident", [P, P], BF, kind="ExternalInput").ap()
    out_p = nc.dram_tensor("out_p", [S, D], BF, kind="ExternalOutput").ap()

    xT_r = xT.rearrange("(kt p) m -> p kt m", p=P)
    out_r = out_p.rearrange("(nt p) d -> p nt d", p=P)

    with tile.TileContext(nc) as tc:
        with (
            tc.tile_pool(name="singles", bufs=1) as singles,
            tc.tile_pool(name="xc_pool", bufs=3) as xpool,
            tc.tile_pool(name="p_pool", bufs=6) as ppool,
            tc.tile_pool(name="vt_pool", bufs=2) as vtpool,
            tc.tile_pool(name="dst_pool", bufs=4) as dstp,
            tc.tile_pool(name="r2_pool", bufs=4) as r2p,
            tc.tile_pool(name="rr_pool", bufs=2) as rrp,
            tc.tile_pool(name="outc_pool", bufs=2) as outcp,
            tc.tile_pool(name="ps_s", bufs=1, space="PSUM") as ps2,
            tc.tile_pool(name="ps_av", bufs=1, space="PSUM") as avp,
            tc.tile_pool(name="ps_aux", bufs=2, space="PSUM") as auxp,
        ):
            qT_sb = singles.tile([P, S], BF)
            kT_sb = singles.tile([P, S], BF)
            oT_sb = singles.tile([P, S], BF)
            # v blocks per m-tile: [v_h0 | ones | v_h1 | ones] (65-wide lhsTs)
            v_sb = singles.tile([P, NT, 130], BF)
            wq_sb = singles.tile([P, KT, P], BF)
            wk_sb = singles.tile([P, KT, P], BF)
            wv_sb = singles.tile([P, KT, P], BF)
            wp_sb = singles.tile([P, D], BF)
            mask_dup = singles.tile([P, 2, P], BF)
            e01_sb = singles.tile([2, P], BF)
            id_sb = singles.tile([P, P], BF)

            nc.sync.dma_start(out=wq_sb, in_=wq.rearrange("(kt p) n -> p kt n", p=P))
            nc.sync.dma_start(out=wk_sb, in_=wk.rearrange("(kt p) n -> p kt n", p=P))
            nc.sync.dma_start(out=wv_sb, in_=wv.rearrange("(kt p) n -> p kt n", p=P))
            nc.sync.dma_start(out=wp_sb, in_=wp)
            nc.sync.dma_start(out=mask_dup[:, 0], in_=maskt)
            nc.sync.dma_start(out=mask_dup[:, 1], in_=maskt)
            nc.sync.dma_start(out=e01_sb, in_=e01)
            nc.sync.dma_start(out=id_sb, in_=ident)
            nc.vector.memset(v_sb[:, :, 64:65], 1.0)
            nc.vector.memset(v_sb[:, :, 129:130], 1.0)

            # ---------------- QKV projection stream (one batch) -----------
            def load_xc(mc):
                sl = slice(mc * 512, (mc + 1) * 512)
                xc = xpool.tile([P, KT, 512], BF, name="xc")
                nc.sync.dma_start(out=xc, in_=xT_r[:, :, sl])
                return xc

            def qkv_stream(b, xc0):
                xcs = xc0
                for ci in range(NCH):
                    mc = b * NCH + ci
                    sl = slice(mc * 512, (mc + 1) * 512)
                    xc = xcs
                    if ci + 1 < NCH:
                        xcs = load_xc(mc + 1)
                    yield
                    for w_sb, dstT in ((wq_sb, qT_sb), (wk_sb, kT_sb)):
                        ps = auxp.tile([P, 512], F32, name="qk_ps", tag="aux")
                        for kt in range(KT):
                            nc.tensor.matmul(ps, lhsT=w_sb[:, kt], rhs=xc[:, kt],
                                             start=(kt == 0), stop=(kt == KT - 1))
                        nc.vector.tensor_copy(out=dstT[:, sl], in_=ps)
                        yield
                    ps_v = auxp.tile([P, 512], F32, name="v_ps", tag="aux")
                    for kt in range(KT):
                        nc.tensor.matmul(ps_v, lhsT=wv_sb[:, kt], rhs=xc[:, kt],
                                         start=(kt == 0), stop=(kt == KT - 1))
                    vt = vtpool.tile([P, 512], BF, name="vt")
                    nc.vector.tensor_copy(out=vt, in_=ps_v)
                    yield
                    for i in range(4):
                        mt = mc * 4 + i
                        ps_t = auxp.tile([P, P], BF, name="ps_t", tag="aux")
                        nc.tensor.transpose(ps_t, vt[:, i * P:(i + 1) * P], id_sb)
                        # [h0(64) | h1(64)] -> v_sb cols [0:64] and [65:129]
                        dst = v_sb[:, mt, 0:130].rearrange(
                            "p (two s) -> p two s", two=2)[:, :, 0:64]
                        src = ps_t.rearrange("p (two s) -> p two s", two=2)
                        nc.vector.tensor_copy(out=dst, in_=src)
                        if i % 2 == 1:
                            yield

            # ---------------- normalize + out-projection (one chunk) ------
            def proj_chunk(b, c, dstages):
                rrbs = []
                for h in (0, 1):
                    rr = rrp.tile([1, 512], F32, name=f"rr{h}")
                    nc.vector.reciprocal_approx_fast(out=rr, in_=dstages[h])
                    rrb = rrp.tile([1, 512], BF, name=f"rrb{h}")
                    nc.vector.tensor_copy(out=rrb, in_=rr)
                    rrbs.append(rrb)
                yield
                # two K=1 broadcast matmuls into disjoint 64-col PE groups
                # (concurrent row/col tiles; e01 row 0 is all-ones over [0:64])
                bc_ps = auxp.tile([P, 512], F32, name="bc", tag="aux")
                for h in (0, 1):
                    nc.tensor.matmul(bc_ps[h * 64:(h + 1) * 64, :],
                                     lhsT=e01_sb[0:1, 0:64], rhs=rrbs[h],
                                     start=True, stop=True)
                sl = slice(b * T + c * 512, b * T + (c + 1) * 512)
                nc.vector.tensor_mul(out=oT_sb[:, sl], in0=oT_sb[:, sl],
                                     in1=bc_ps)
                outc = outcp.tile([P, 4 * D], BF, name="outc")
                yield
                for i in range(4):
                    mt = (b * T + c * 512) // P + i
                    for nch in range(2):
                        pj = auxp.tile([P, 512], F32, name="pj", tag="aux")
                        nc.tensor.matmul(
                            pj, lhsT=oT_sb[:, mt * P:(mt + 1) * P],
                            rhs=wp_sb[:, nch * 512:(nch + 1) * 512],
                            start=True, stop=True)
                        eng = nc.scalar if (b == B - 1 and c == NCH - 1
                                            and nch == 1) else None
                        dst_ap = outc[:, i * D + nch * 512: i * D + (nch + 1) * 512]
                        if eng is None:
                            nc.vector.tensor_copy(out=dst_ap, in_=pj)
                        else:
                            nc.scalar.copy(out=dst_ap, in_=pj)
                    nc.sync.dma_start(
                        out=out_r[:, mt, :],
                        in_=outc[:, i * D:(i + 1) * D])
                    yield

            # ---------------- attention stream (one batch) -----------------
            def attn_stream(b, on_chunk_done, on_chunk_start=None):
                for c in range(NCH):
                    if on_chunk_start is not None:
                        on_chunk_start(b, c)
                    av_t = [avp.tile([P, 512], F32, name=f"av_{h}")
                            for h in (0, 1)]
                    dstages = []
                    pending = None
                    n_jt = 4 * c + 4

                    def off_of(jt):
                        return jt * P - c * 512 if jt // 4 == c else 0

                    def flush_av():
                        nonlocal pending
                        if pending is None:
                            return
                        j0, p_sb = pending
                        for idx in (0, 1):
                            jt = j0 + idx
                            off = off_of(jt)
                            for h in (0, 1):
                                lv = v_sb[:, b * JT + jt, 65 * h: 65 * h + 65]
                                nc.tensor.matmul(
                                    av_t[h][0:65, off:512],
                                    lhsT=lv,
                                    rhs=p_sb[:, 1024 * idx + 512 * h + off:
                                             1024 * idx + 512 * (h + 1)],
                                    start=(jt == 0), stop=(jt == n_jt - 1))
                        pending = None

                    # two j-tiles per step: one 4-bank score tile, ONE exp
                    # instruction for all four head-blocks (each ACTIVATE
                    # pays a ~352-cycle pipeline fill, so fewer+wider wins)
                    for j0 in range(0, n_jt, 2):
                        flush_av()
                        s_ps = ps2.tile([P, 2048], F32, name="s_ps")
                        for idx in (0, 1):
                            jt = j0 + idx
                            off = off_of(jt)
                            for h in (0, 1):
                                lk = kT_sb[h * 64:(h + 1) * 64,
                                           b * T + jt * P: b * T + (jt + 1) * P]
                                rq = qT_sb[h * 64:(h + 1) * 64,
                                           b * T + c * 512 + off: b * T + (c + 1) * 512]
                                nc.tensor.matmul(
                                    s_ps[:, 1024 * idx + 512 * h + off:
                                         1024 * idx + 512 * (h + 1)],
                                    lhsT=lk, rhs=rq, start=True, stop=True)
                        p_sb = ppool.tile([P, 2048], BF, name="p_sb")
                        off0 = off_of(j0)
                        nc.scalar.activation(
                            out=p_sb[:, off0:2048], in_=s_ps[:, off0:2048],
                            func=Exp, scale=SCALE)
                        for idx in (0, 1):
                            jt = j0 + idx
                            if jt // 4 == c:
                                off = off_of(jt)
                                pd = p_sb[:, 1024 * idx: 1024 * (idx + 1)].rearrange(
                                    "p (two s) -> p two s", s=512)[:, :, off:off + P]
                                nc.gpsimd.tensor_mul(out=pd, in0=pd, in1=mask2_sb)
                        pending = (j0, p_sb)
                        yield
                    flush_av()
                    # oT (unnormalized) + denominator rows -> r2c via DMA
                    # (engine APs need 32-aligned partition base, so rows 0/1
                    # of r2c are written by DMA scatter from a staging row)
                    csl = slice(b * T + c * 512, b * T + (c + 1) * 512)
                    for h in (0, 1):
                        nc.vector.tensor_copy(
                            out=oT_sb[h * 64:(h + 1) * 64, csl],
                            in_=av_t[h][0:64])
                        dstage = dstp.tile([1, 512], F32, name=f"dstage{h}")
                        nc.vector.tensor_copy(out=dstage, in_=av_t[h][64:65])
                        dstages.append(dstage)
                    on_chunk_done(b, c, tuple(dstages))
                    dstages = []
                    yield

            # ---------------- interleaved schedule -------------------------
            fillers = deque()

            def pump():
                while fillers:
                    g = fillers.popleft()
                    try:
                        next(g)
                    except StopIteration:
                        continue
                    fillers.append(g)
                    return True
                return False

            def drain(g):
                for _ in g:
                    pass

            def on_chunk_done(b, c, r2c):
                fillers.append(proj_chunk(b, c, r2c))

            mask2_sb = mask_dup[:, :, :]

            # warm the PE / HAM clock gate while initial DMAs land
            wu_sb = singles.tile([P, P], BF)
            nc.vector.memset(wu_sb, 0.0)
            for _ in range(48):
                wu_ps = auxp.tile([P, P], F32, name="wu", tag="aux")
                nc.tensor.matmul(wu_ps, lhsT=wu_sb, rhs=wu_sb,
                                 start=True, stop=True)

            g0 = qkv_stream(0, load_xc(0))
            g0_done = [0]

            def pump_g0(n):
                # qkv(0) chunk c is fully emitted after 6*(c+1) quanta;
                # attention(0, c) must not be emitted before it (deps are
                # tracked in emission order)
                while g0_done[0] < n:
                    if next(g0, StopIteration) is StopIteration:
                        g0_done[0] = 10 ** 9
                        return
                    g0_done[0] += 1

            def on_chunk_start(b, c):
                if b == 0:
                    pump_g0(6 * (c + 1))

            pump_g0(6)
            fillers.append(g0)
            for b in range(B):
                if b + 1 < B:
                    fillers.append(qkv_stream(b + 1, load_xc((b + 1) * NCH)))
                for _ in attn_stream(b, on_chunk_done,
                                     on_chunk_start if b == 0 else None):
                    pump()
                    pump()
            while pump():
                pass

    nc.compile()
    return nc


def _host_inputs(x, w_qkv, w_proj):
    x = np.asarray(x, dtype=np.float32)
    w_qkv = np.asarray(w_qkv, dtype=np.float32)
    w_proj = np.asarray(w_proj, dtype=np.float32)

    xT = np.ascontiguousarray(x.reshape(S, D).T).astype(BFNP)
    mask = np.triu(np.ones((P, P), np.float32)).astype(BFNP)  # [j, i]: 1 if j<=i
    e01 = np.zeros((2, P), np.float32)
    e01[0, :64] = 1.0
    e01[1, 64:] = 1.0
    e01 = e01.astype(BFNP)
    ident = np.eye(P, dtype=np.float32).astype(BFNP)

    in_maps = []
    for core in range(N_CORES):
        cs = slice(core * P, (core + 1) * P)
        in_maps.append({
            "xT": xT,
            "wq": np.ascontiguousarray(w_qkv[:, core * P:(core + 1) * P]).astype(BFNP),
            "wk": np.ascontiguousarray(w_qkv[:, D + core * P: D + (core + 1) * P]).astype(BFNP),
            "wv": np.ascontiguousarray(w_qkv[:, 2 * D + core * P: 2 * D + (core + 1) * P]).astype(BFNP),
            "wp": np.ascontiguousarray(w_proj[cs, :]).astype(BFNP),
            "maskt": mask,
            "e01": e01,
            "ident": ident,
        })
    return in_maps


def run_spmd(x, w_qkv, w_proj, trace=False):
    """Compile (cached) + run on 8 cores. Returns (out [B,T,D] fp32, results)."""
    from concourse import bass_utils

    if "nc" not in _CACHE:
        _CACHE["nc"] = _build_nc()
    nc = _CACHE["nc"]

    in_maps = _host_inputs(x, w_qkv, w_proj)
    res = bass_utils.run_bass_kernel_spmd(
        nc, in_maps, core_ids=list(range(N_CORES)), trace=trace)

    acc = np.zeros((S, D), np.float32)
    for r in res.results:
        acc += np.asarray(r["out_p"]).astype(np.float32)
    return acc.reshape(B, T, D), res


def kernel(x, w_qkv, w_proj):
    out, _ = run_spmd(x, w_qkv, w_proj, trace=False)
    return out


# revision 20
# speedup vs baseline: 1.1999x; 1.1999x over previous
"""Causal self-attention (B=4, T=2048, D=1024, H=16) on 8 trn2 NeuronCores.

Sharding: tensor-parallel over heads — 2 heads per core. Each core computes
qkv projections for its 2 heads (from replicated x), causal attention, and a
partial output projection (its 128 rows of w_proj). Host sums the 8 partial
[S, D] outputs.

v2: fine-grained software pipeline. The attention inner loop is ScalarE
latency-bound (exp of a [128,1024] score tile ~1.15us vs ~0.85us of PE work
per j-tile), so QKV-projection matmuls of batch b+1 and output-projection
matmuls of chunk (b,c-1) are interleaved between attention steps of batch b
at ~1-matmul-group granularity. This keeps TensorE busy through the
scalar-bound stretches, avoids >3.4us PE idle windows (which drop the HAM
clock gate from 2.4GHz to 1.2GHz), and removes the serial per-batch
normalize+project tail.

Other changes vs v1:
  - diagonal score matmuls N-sliced to the causal region (rhs[off:])
  - per-chunk softmax denominators: per-head DVE reciprocal_approx_fast on
    the [1,512] staging rows (exp/recip can't share the ScalarE table set,
    and InstReciprocal is 5x slower), then two K=1 broadcast matmuls into
    disjoint 64-col PE groups (they pair concurrently, like the K=64
    score-matmul pairs in PE row groups 0/64)
  - causal masks as one strided 2-head gpsimd multiply per diagonal tile
    (gpsimd's dedicated queue keeps them off the DVE/ScalarE critical paths)
  - x chunks prefetched one ahead; per-chunk coalesced x loads; per-tile
    output stores; PE warm-up matmuls cover the initial DMA latency
"""

import math
from collections import deque

import numpy as np
import ml_dtypes

B, T, D, H = 4, 2048, 1024, 16
HD = D // H           # 64
S = B * T             # 8192
P = 128
KT = D // P           # 8 k-tiles over D
NT = S // P           # 64 m-tiles of 128
JT = T // P           # 16 j-tiles per batch
NCH = T // 512        # 4 i-chunks per batch
N_CORES = 8

BFNP = ml_dtypes.bfloat16

_CACHE = {}


def _build_nc():
    import concourse.tile as tile
    import concourse.mybir as mybir
    from concourse import bacc

    BF = mybir.dt.bfloat16
    F32 = mybir.dt.float32
    F32R = mybir.dt.float32r
    Exp = mybir.ActivationFunctionType.Exp
    SCALE = 1.0 / math.sqrt(HD)

    nc = bacc.Bacc("TRN2", num_devices=N_CORES)

    xT = nc.dram_tensor("xT", [D, S], BF, kind="ExternalInput").ap()
    wq = nc.dram_tensor("wq", [D, P], BF, kind="ExternalInput").ap()
    wk = nc.dram_tensor("wk", [D, P], BF, kind="ExternalInput").ap()
    wv = nc.dram_tensor("wv", [D, P], BF, kind="ExternalInput").ap()
    wp = nc.dram_tensor("wp", [P, D], BF, kind="ExternalInput").ap()
    maskt = nc.dram_tensor("maskt", [P, P], BF, kind="ExternalInput").ap()
    e01 = nc.dram_tensor("e01", [2, P], BF, kind="ExternalInput").ap()
    ident = nc.dram_tensor("ident", [P, P], BF, kind="ExternalInput").ap()
    out_p = nc.dram_tensor("out_p", [S, D], BF, kind="ExternalOutput").ap()

    xT_r = xT.rearrange("(kt p) m -> p kt m", p=P)
    out_r = out_p.rearrange("(nt p) d -> p nt d", p=P)

    with tile.TileContext(nc) as tc:
        with (
            tc.tile_pool(name="singles", bufs=1) as singles,
            tc.tile_pool(name="xc_pool", bufs=3) as xpool,
            tc.tile_pool(name="p_pool", bufs=6) as ppool,
            tc.tile_pool(name="vt_pool", bufs=2) as vtpool,
            tc.tile_pool(name="dst_pool", bufs=4) as dstp,
            tc.tile_pool(name="r2_pool", bufs=4) as r2p,
            tc.tile_pool(name="rr_pool", bufs=2) as rrp,
            tc.tile_pool(name="outc_pool", bufs=2) as outcp,
            tc.tile_pool(name="ps_s", bufs=2, space="PSUM") as ps2,
            tc.tile_pool(name="ps_av", bufs=1, space="PSUM") as avp,
            tc.tile_pool(name="ps_aux", bufs=2, space="PSUM") as auxp,
        ):
            qT_sb = singles.tile([P, S], BF)
            kT_sb = singles.tile([P, S], BF)
            oT_sb = singles.tile([P, S], BF)
            # v blocks per m-tile: [v_h0 | ones | v_h1 | ones] (65-wide lhsTs)
            v_sb = singles.tile([P, NT, 130], BF)
            wq_sb = singles.tile([P, KT, P], BF)
            wk_sb = singles.tile([P, KT, P], BF)
            wv_sb = singles.tile([P, KT, P], BF)
            wp_sb = singles.tile([P, D], BF)
            mask_dup = singles.tile([P, 2, P], BF)
            e01_sb = singles.tile([2, P], BF)
            id_sb = singles.tile([P, P], BF)

            nc.sync.dma_start(out=wq_sb, in_=wq.rearrange("(kt p) n -> p kt n", p=P))
            nc.sync.dma_start(out=wk_sb, in_=wk.rearrange("(kt p) n -> p kt n", p=P))
            nc.sync.dma_start(out=wv_sb, in_=wv.rearrange("(kt p) n -> p kt n", p=P))
            nc.sync.dma_start(out=wp_sb, in_=wp)
            nc.sync.dma_start(out=mask_dup[:, 0], in_=maskt)
            nc.sync.dma_start(out=mask_dup[:, 1], in_=maskt)
            nc.sync.dma_start(out=e01_sb, in_=e01)
            nc.sync.dma_start(out=id_sb, in_=ident)
            nc.vector.memset(v_sb[:, :, 64:65], 1.0)
            nc.vector.memset(v_sb[:, :, 129:130], 1.0)

            # ---------------- QKV projection stream (one batch) -----------
            def load_xc(mc):
                sl = slice(mc * 512, (mc + 1) * 512)
                xc = xpool.tile([P, KT, 512], BF, name="xc")
                nc.sync.dma_start(out=xc, in_=xT_r[:, :, sl])
                return xc

            def qkv_stream(b, xc0):
                xcs = xc0
                for ci in range(NCH):
                    mc = b * NCH + ci
                    sl = slice(mc * 512, (mc + 1) * 512)
                    xc = xcs
                    if ci + 1 < NCH:
                        xcs = load_xc(mc + 1)
                    yield
                    for w_sb, dstT in ((wq_sb, qT_sb), (wk_sb, kT_sb)):
                        ps = auxp.tile([P, 512], F32, name="qk_ps", tag="aux")
                        for kt in range(KT):
                            nc.tensor.matmul(ps, lhsT=w_sb[:, kt], rhs=xc[:, kt],
                                             start=(kt == 0), stop=(kt == KT - 1))
                        nc.vector.tensor_copy(out=dstT[:, sl], in_=ps)
                        yield
                    ps_v = auxp.tile([P, 512], F32, name="v_ps", tag="aux")
                    for kt in range(KT):
                        nc.tensor.matmul(ps_v, lhsT=wv_sb[:, kt], rhs=xc[:, kt],
                                         start=(kt == 0), stop=(kt == KT - 1))
                    vt = vtpool.tile([P, 512], BF, name="vt")
                    nc.vector.tensor_copy(out=vt, in_=ps_v)
                    yield
                    for i in range(4):
                        mt = mc * 4 + i
                        ps_t = auxp.tile([P, P], BF, name="ps_t", tag="aux")
                        nc.tensor.transpose(ps_t, vt[:, i * P:(i + 1) * P], id_sb)
                        # [h0(64) | h1(64)] -> v_sb cols [0:64] and [65:129]
                        dst = v_sb[:, mt, 0:130].rearrange(
                            "p (two s) -> p two s", two=2)[:, :, 0:64]
                        src = ps_t.rearrange("p (two s) -> p two s", two=2)
                        nc.vector.tensor_copy(out=dst, in_=src)
                        if i % 2 == 1:
                            yield

            # ---------------- normalize + out-projection (one chunk) ------
            def proj_chunk(b, c, dstages):
                rrbs = []
                for h in (0, 1):
                    rr = rrp.tile([1, 512], F32, name=f"rr{h}")
                    nc.vector.reciprocal_approx_fast(out=rr, in_=dstages[h])
                    rrb = rrp.tile([1, 512], BF, name=f"rrb{h}")
                    nc.vector.tensor_copy(out=rrb, in_=rr)
                    rrbs.append(rrb)
                yield
                # two K=1 broadcast matmuls into disjoint 64-col PE groups
                # (concurrent row/col tiles; e01 row 0 is all-ones over [0:64])
                bc_ps = auxp.tile([P, 512], F32, name="bc", tag="aux")
                for h in (0, 1):
                    nc.tensor.matmul(bc_ps[h * 64:(h + 1) * 64, :],
                                     lhsT=e01_sb[0:1, 0:64], rhs=rrbs[h],
                                     start=True, stop=True)
                sl = slice(b * T + c * 512, b * T + (c + 1) * 512)
                nc.vector.tensor_mul(out=oT_sb[:, sl], in0=oT_sb[:, sl],
                                     in1=bc_ps)
                outc = outcp.tile([P, 4 * D], BF, name="outc")
                yield
                for i in range(4):
                    mt = (b * T + c * 512) // P + i
                    for nch in range(2):
                        pj = auxp.tile([P, 512], F32, name="pj", tag="aux")
                        nc.tensor.matmul(
                            pj, lhsT=oT_sb[:, mt * P:(mt + 1) * P],
                            rhs=wp_sb[:, nch * 512:(nch + 1) * 512],
                            start=True, stop=True)
                        eng = nc.scalar if (b == B - 1 and c == NCH - 1
                                            and nch == 1) else None
                        dst_ap = outc[:, i * D + nch * 512: i * D + (nch + 1) * 512]
                        if eng is None:
                            nc.vector.tensor_copy(out=dst_ap, in_=pj)
                        else:
                            nc.scalar.copy(out=dst_ap, in_=pj)
                    nc.sync.dma_start(
                        out=out_r[:, mt, :],
                        in_=outc[:, i * D:(i + 1) * D])
                    yield

            # ---------------- attention stream (one batch) -----------------
            def attn_stream(b, on_chunk_done, on_chunk_start=None):
                for c in range(NCH):
                    if on_chunk_start is not None:
                        on_chunk_start(b, c)
                    av_t = [avp.tile([P, 512], F32, name=f"av_{h}")
                            for h in (0, 1)]
                    dstages = []
                    pending = None
                    n_jt = 4 * c + 4

                    def flush_av():
                        nonlocal pending
                        if pending is None:
                            return
                        jt, p_sb, off = pending
                        for h in (0, 1):
                            lv = v_sb[:, b * JT + jt, 65 * h: 65 * h + 65]
                            nc.tensor.matmul(
                                av_t[h][0:65, off:512],
                                lhsT=lv,
                                rhs=p_sb[:, 512 * h + off: 512 * (h + 1)],
                                start=(jt == 0), stop=(jt == n_jt - 1))
                        pending = None

                    for jt in range(n_jt):
                        diag = (jt // 4 == c)
                        off = jt * P - c * 512 if diag else 0
                        flush_av()
                        s_ps = ps2.tile([P, 1024], F32, name="s_ps")
                        for h in (0, 1):
                            lk = kT_sb[h * 64:(h + 1) * 64,
                                       b * T + jt * P: b * T + (jt + 1) * P]
                            rq = qT_sb[h * 64:(h + 1) * 64,
                                       b * T + c * 512 + off: b * T + (c + 1) * 512]
                            nc.tensor.matmul(
                                s_ps[:, 512 * h + off: 512 * (h + 1)],
                                lhsT=lk, rhs=rq, start=True, stop=True)
                        p_sb = ppool.tile([P, 1024], BF, name="p_sb")
                        # single exp over both heads (junk span between the
                        # halves is never read downstream)
                        nc.scalar.activation(
                            out=p_sb[:, off:1024], in_=s_ps[:, off:1024],
                            func=Exp, scale=SCALE)
                        if diag:
                            # both heads' diag blocks sit 512 apart: one
                            # strided gpsimd multiply (own queue, off the
                            # DVE critical path)
                            pd = p_sb.rearrange(
                                "p (two s) -> p two s", s=512)[:, :, off:off + P]
                            nc.gpsimd.tensor_mul(out=pd, in0=pd, in1=mask2_sb)
                        pending = (jt, p_sb, off)
                        yield
                    flush_av()
                    # oT (unnormalized) + denominator rows -> r2c via DMA
                    # (engine APs need 32-aligned partition base, so rows 0/1
                    # of r2c are written by DMA scatter from a staging row)
                    csl = slice(b * T + c * 512, b * T + (c + 1) * 512)
                    for h in (0, 1):
                        nc.vector.tensor_copy(
                            out=oT_sb[h * 64:(h + 1) * 64, csl],
                            in_=av_t[h][0:64])
                        dstage = dstp.tile([1, 512], F32, name=f"dstage{h}")
                        nc.vector.tensor_copy(out=dstage, in_=av_t[h][64:65])
                        dstages.append(dstage)
                    on_chunk_done(b, c, tuple(dstages))
                    dstages = []
                    yield

            # ---------------- interleaved schedule -------------------------
            fillers = deque()

            def pump():
                while fillers:
                    g = fillers.popleft()
                    try:
                        next(g)
                    except StopIteration:
                        continue
                    fillers.append(g)
                    return True
                return False

            def drain(g):
                for _ in g:
                    pass

            def on_chunk_done(b, c, r2c):
                fillers.append(proj_chunk(b, c, r2c))

            mask2_sb = mask_dup[:, :, :]

            # warm the PE / HAM clock gate while initial DMAs land
            wu_sb = singles.tile([P, P], BF)
            nc.vector.memset(wu_sb, 0.0)
            for _ in range(48):
                wu_ps = auxp.tile([P, P], F32, name="wu", tag="aux")
                nc.tensor.matmul(wu_ps, lhsT=wu_sb, rhs=wu_sb,
                                 start=True, stop=True)

            g0 = qkv_stream(0, load_xc(0))
            g0_done = [0]

            def pump_g0(n):
                # qkv(0) chunk c is fully emitted after 6*(c+1) quanta;
                # attention(0, c) must not be emitted before it (deps are
                # tracked in emission order)
                while g0_done[0] < n:
                    if next(g0, StopIteration) is StopIteration:
                        g0_done[0] = 10 ** 9
                        return
                    g0_done[0] += 1

            def on_chunk_start(b, c):
                if b == 0:
                    pump_g0(6 * (c + 1))

            pump_g0(6)
            fillers.append(g0)
            for b in range(B):
                if b + 1 < B:
                    fillers.append(qkv_stream(b + 1, load_xc((b + 1) * NCH)))
                for _ in attn_stream(b, on_chunk_done,
                                     on_chunk_start if b == 0 else None):
                    pump()
            while pump():
                pass

    nc.compile()
    return nc


def _host_inputs(x, w_qkv, w_proj):
    x = np.asarray(x, dtype=np.float32)
    w_qkv = np.asarray(w_qkv, dtype=np.float32)
    w_proj = np.asarray(w_proj, dtype=np.float32)

    xT = np.ascontiguousarray(x.reshape(S, D).T).astype(BFNP)
    mask = np.triu(np.ones((P, P), np.float32)).astype(BFNP)  # [j, i]: 1 if j<=i
    e01 = np.zeros((2, P), np.float32)
    e01[0, :64] = 1.0
    e01[1, 64:] = 1.0
    e01 = e01.astype(BFNP)
    ident = np.eye(P, dtype=np.float32).astype(BFNP)

    in_maps = []
    for core in range(N_CORES):
        cs = slice(core * P, (core + 1) * P)
        in_maps.append({
            "xT": xT,
            "wq": np.ascontiguousarray(w_qkv[:, core * P:(core + 1) * P]).astype(BFNP),
            "wk": np.ascontiguousarray(w_qkv[:, D + core * P: D + (core + 1) * P]).astype(BFNP),
            "wv": np.ascontiguousarray(w_qkv[:, 2 * D + core * P: 2 * D + (core + 1) * P]).astype(BFNP),
            "wp": np.ascontiguousarray(w_proj[cs, :]).astype(BFNP),
            "maskt": mask,
            "e01": e01,
            "ident": ident,
        })
    return in_maps


def run_spmd(x, w_qkv, w_proj, trace=False):
    """Compile (cached) + run on 8 cores. Returns (out [B,T,D] fp32, results)."""
    from concourse import bass_utils

    if "nc" not in _CACHE:
        _CACHE["nc"] = _build_nc()
    nc = _CACHE["nc"]

    in_maps = _host_inputs(x, w_qkv, w_proj)
    res = bass_utils.run_bass_kernel_spmd(
        nc, in_maps, core_ids=list(range(N_CORES)), trace=trace)

    acc = np.zeros((S, D), np.float32)
    for r in res.results:
        acc += np.asarray(r["out_p"]).astype(np.float32)
    return acc.reshape(B, T, D), res


def kernel(x, w_qkv, w_proj):
    out, _ = run_spmd(x, w_qkv, w_proj, trace=False)
    return out
